# revision 28
# baseline (speedup 1.0000x reference)
"""Bahdanau additive attention (vectorized) on TRN2 — Bass/Tile kernel.

Problem: nn_AttentionLayer_11055245820581
  e[b,y,x] = softmax_x( sum_e V[e] * tanh(Ws[b,x,e] + Uh[b,y,e]) )
  c[b,y,:] = sum_x e[b,y,x] * enc[b,x,:]
with Ws = enc @ W_a, Uh = dec @ U_a.

Sharding: data-parallel over batch B=8 across the 8 NeuronCores (one
batch element per core). Each core computes its batch's full attention.

Per-core dataflow (the tanh cube Ty*Tx*E = 16.7M elements dominates;
ACT's 1 elem/lane/cycle tanh is the ~110us floor, everything else is
arranged to stay below it):
  - broadcast-add WsT[e,x] + UhT[e,y] into fp16 slabs, split per block
    between DVE (tensor_scalar_add, per-partition scalar) and GPSIMD
    (one tensor_tensor with step-0 broadcast APs per 32-y slab) so
    neither engine exceeds ACT's per-block budget.
  - ACT: one big Tanh per (y-block, chunk) slab -> fp16 tanh slab.
  - PE: projection with tanh slab as fp16 stationary [128e, 128x] and
    V fp16 moving: e'^T lands as [x(partition), y] columns in PSUM
    (M=128 amortizes the per-matmul fixed cost; no PSUM evacuation).
  - softmax in the transposed layout: ACT Exp -> expT in SBUF; row sums
    over x via matmul with a ones vector -> denom[y]; DVE reciprocal;
    context matmul uses unnormalized expT and scales c rows by 1/denom;
    attention weights are PE-transposed back to [y, x] and scaled.
"""

import numpy as np
from contextlib import ExitStack

import concourse.bass as bass
import concourse.bacc as bacc
import concourse.tile as tile
from concourse import mybir
from concourse.bass_utils import run_bass_kernel_spmd

B, Tx, Ty, E, D = 8, 256, 256, 256, 256
P = 128
NCORES = 8
F32 = mybir.dt.float32
F16 = mybir.dt.float16
TANH = mybir.ActivationFunctionType.Tanh
EXP = mybir.ActivationFunctionType.Exp

G = 32           # y-block size of the main loop
NB = Ty // G     # 8 blocks
EC = E // P      # 2 e-chunks
XC = Tx // P     # 2 x-chunks
YC = Ty // P     # 2 y-halves
DC = D // P      # 2 d-chunks

_NC = None
LAST_RESULTS = None


def _bcast_add_ap(t, n_rep, n_inner):
    """AP reading a [P, n_inner] tile as [P, n_rep, n_inner] (repeat dim 1)."""
    return bass.AP(tensor=t.tensor, offset=t.offset,
                   ap=[t.ap[0], [0, n_rep], t.ap[1]])


def _bcast_inner_ap(t, col0, n_rep, n_inner):
    """AP reading tile columns [col0:col0+n_rep] as [P, n_rep, n_inner]
    (each column repeated n_inner times along the innermost dim)."""
    step = t.ap[1][0]
    return bass.AP(tensor=t.tensor, offset=t.offset + col0 * step,
                   ap=[t.ap[0], [step, n_rep], [0, n_inner]])


def _build_body(tc, ctx, enc_d, dec_d, W_d, U_d, V_d, c_d, e_d):
    nc = tc.nc
    from concourse.masks import make_identity

    consts = ctx.enter_context(tc.tile_pool(name="consts", bufs=1))
    add_pool = ctx.enter_context(tc.tile_pool(name="adds", bufs=3))
    tanh_pool = ctx.enter_context(tc.tile_pool(name="tanhs", bufs=3))
    out_pool = ctx.enter_context(tc.tile_pool(name="outs", bufs=2))
    misc_psum = ctx.enter_context(tc.tile_pool(name="pmisc", bufs=2, space="PSUM"))
    e_psum = ctx.enter_context(tc.tile_pool(name="pe", bufs=1, space="PSUM"))
    piece_psum = ctx.enter_context(tc.tile_pool(name="ppiece", bufs=2, space="PSUM"))

    # ---- load inputs ----
    enc_sb = consts.tile([P, XC, E], F32)    # [x_in_chunk, (xc), e]
    dec_sb = consts.tile([P, YC, D], F32)
    W_sb = consts.tile([P, EC, E], F32)      # rows e_in
    U_sb = consts.tile([P, DC, E], F32)      # rows d
    V_sb = consts.tile([P, EC], F32)
    for i in range(XC):
        nc.sync.dma_start(out=enc_sb[:, i, :], in_=enc_d[i * P:(i + 1) * P, :])
    for i in range(YC):
        nc.sync.dma_start(out=dec_sb[:, i, :], in_=dec_d[i * P:(i + 1) * P, :])
    for i in range(EC):
        nc.sync.dma_start(out=W_sb[:, i, :], in_=W_d[i * P:(i + 1) * P, :])
    for i in range(DC):
        nc.sync.dma_start(out=U_sb[:, i, :], in_=U_d[i * P:(i + 1) * P, :])
    for i in range(EC):
        nc.sync.dma_start(out=V_sb[:, i:i + 1], in_=V_d[i * P:(i + 1) * P, :])

    ident = consts.tile([P, P], F32)
    make_identity(nc, ident)
    ident16 = consts.tile([P, P], F16)
    nc.vector.tensor_copy(ident16[:], ident[:])
    ones_sb = consts.tile([P, 1], F32)
    nc.vector.memset(ones_sb[:], 1.0)
    V16_sb = consts.tile([P, EC], F16)
    nc.vector.tensor_copy(V16_sb[:], V_sb[:])
    # Trigger the ACT tanh table load during the otherwise-idle prologue.
    warm_sb = consts.tile([P, 1], F32)
    nc.scalar.activation(out=warm_sb[:], in_=ones_sb[:], func=TANH)

    # ---- transpose enc, dec (PE transpose via identity) ----
    encT_sb = consts.tile([P, EC, Tx], F32)  # [e, (ec), x]
    decT_sb = consts.tile([P, DC, Ty], F32)  # [d, (dc), y]
    for src, srcC, dstT, dstC in ((enc_sb, XC, encT_sb, EC),
                                  (dec_sb, YC, decT_sb, DC)):
        for i in range(srcC):          # source partition chunk (x or y)
            for j in range(dstC):      # source free chunk (e or d)
                pt = misc_psum.tile([P, Tx], F32, tag="ps", name="pt")
                nc.tensor.transpose(
                    out=pt[:, :P], in_=src[:, i, j * P:(j + 1) * P],
                    identity=ident[:])
                nc.vector.tensor_copy(dstT[:, j, i * P:(i + 1) * P], pt[:, :P])

    # ---- WsT[e_out, x] = sum_ei W[ei, e_out] * encT[ei, x] ----
    # fp16 copies feed the DVE/GPSIMD adds; fp32 UhT feeds the DVE
    # per-partition scalar reads (TensorScalar requires fp32 scalars).
    WsT16_sb = consts.tile([P, EC, Tx], F16)
    UhT16_sb = consts.tile([P, EC, Ty], F16)
    UhT_sb = consts.tile([P, EC, Ty], F32)
    for co in range(EC):
        pw = misc_psum.tile([P, Tx], F32, tag="ps", name="pw")
        for ci in range(EC):
            nc.tensor.matmul(
                out=pw[:], lhsT=W_sb[:, ci, co * P:(co + 1) * P],
                rhs=encT_sb[:, ci, :], start=(ci == 0), stop=(ci == EC - 1))
        nc.vector.tensor_copy(WsT16_sb[:, co, :], pw[:])
    for co in range(EC):
        pu = misc_psum.tile([P, Ty], F32, tag="ps", name="pu")
        for ci in range(DC):
            nc.tensor.matmul(
                out=pu[:], lhsT=U_sb[:, ci, co * P:(co + 1) * P],
                rhs=decT_sb[:, ci, :], start=(ci == 0), stop=(ci == DC - 1))
        nc.vector.tensor_copy(UhT_sb[:, co, :], pu[:])
        nc.vector.tensor_copy(UhT16_sb[:, co, :], pu[:])

    # ---- main loop: tanh cube + V projection into e'^T ----
    # e'^T[x, (xc, y)] accumulates into one [128, XC*128] PSUM tile per
    # y-half (1 bank each) so each half's softmax can start while the
    # other half is still being produced.
    eT_yh = [e_psum.tile([P, XC, P], F32, tag=f"e{h}", name=f"eT_yh{h}")
             for h in range(YC)]
    for h in range(YC):
        nc.vector.memset(eT_yh[h][:], 0.0)

    # ---- per-y-half softmax + context + attention-weight output ----
    expT_sb = consts.tile([P, XC, Ty], F32)  # [x, (xc), y]
    recip_sb = consts.tile([P, YC], F32)
    alpha_sb = consts.tile([P, YC, Tx], F32)

    def _final_half(yh):
        for xc in range(XC):
            nc.scalar.activation(out=expT_sb[:, xc, yh * P:(yh + 1) * P],
                                 in_=eT_yh[yh][:, xc, :], func=EXP)
        den = misc_psum.tile([P, 1], F32, tag="ps", name=f"den{yh}")
        for xc in range(XC):
            nc.tensor.matmul(
                out=den[:],
                lhsT=expT_sb[:, xc, yh * P:(yh + 1) * P],
                rhs=ones_sb[:],
                start=(xc == 0), stop=(xc == XC - 1))
        nc.vector.reciprocal(recip_sb[:, yh:yh + 1], den[:])
        pc = misc_psum.tile([P, E], F32, tag="ps", name=f"pc{yh}")
        for xc in range(XC):
            nc.tensor.matmul(
                out=pc[:], lhsT=expT_sb[:, xc, yh * P:(yh + 1) * P],
                rhs=enc_sb[:, xc, :], start=(xc == 0), stop=(xc == XC - 1))
        c_sb = out_pool.tile([P, E], F32, tag="c_sb", name=f"c_sb{yh}")
        nc.vector.tensor_scalar_mul(
            out=c_sb[:], in0=pc[:], scalar1=recip_sb[:, yh:yh + 1])
        nc.sync.dma_start(out=c_d[yh * P:(yh + 1) * P, :], in_=c_sb[:])
        for xc in range(XC):
            pt2 = misc_psum.tile([P, E], F32, tag="ps", name=f"pt2_{yh}_{xc}")
            nc.tensor.transpose(
                out=pt2[:, :P], in_=expT_sb[:, xc, yh * P:(yh + 1) * P],
                identity=ident[:])
            nc.vector.tensor_scalar_mul(
                out=alpha_sb[:, yh, xc * P:(xc + 1) * P], in0=pt2[:, :P],
                scalar1=recip_sb[:, yh:yh + 1])
        nc.sync.dma_start(out=e_d[yh * P:(yh + 1) * P, :],
                          in_=alpha_sb[:, yh, :])

    # First PY y's of every (block, chunk) get their broadcast-add done on
    # the Tensor engine (identity matmuls of a step-0-broadcast W plus an
    # inner-broadcast U, accumulated bank-by-bank into one PSUM piece),
    # the rest on DVE via fp16 tensor_scalar. ACT tanh-reads the PSUM
    # piece in a single op.
    # Block schedule: PY y's of every (block, chunk) go via the PE piece
    # path (identity matmuls of broadcast W + broadcast U into PSUM
    # halves, double-buffered so ACT never waits on PE), the rest via DVE
    # fp16 tensor_scalar. Small tail blocks keep the drain chain short.
    PY = 8
    blocks = [(48, 8), (48, 8), (32, 8), (48, 8), (48, 8), (16, 8), (16, 8)]
    assert sum(gb for gb, _ in blocks) == Ty
    y0 = 0
    for b, (GB, PY) in enumerate(blocks):
        slabs = []
        for c in range(EC):
            tslab = tanh_pool.tile([P, GB, Tx], F16, tag="tanh",
                                   name=f"tanh{b}_{c}")
            for h in range(PY // 4):
                piece = piece_psum.tile([P, 4 * Tx], F32, tag="piece",
                                        name=f"piece{b}_{c}_{h}")
                yp = y0 + 4 * h
                for s in range(2):
                    sub = piece[:, 2 * Tx * s:2 * Tx * (s + 1)]
                    nc.tensor.matmul(
                        out=sub,
                        lhsT=ident16[:],
                        rhs=_bcast_add_ap(WsT16_sb[:, c, :], 2, Tx),
                        start=True, stop=False)
                    nc.tensor.matmul(
                        out=sub,
                        lhsT=ident16[:],
                        rhs=_bcast_inner_ap(UhT16_sb[:, c, :], yp + 2 * s,
                                            2, Tx),
                        start=False, stop=True)
                nc.scalar.activation(out=tslab[:, 4 * h:4 * h + 4, :],
                                     in_=piece[:], func=TANH)
            ndve = GB - PY
            if ndve:
                aslab = add_pool.tile([P, ndve, Tx], F16, tag="add",
                                      name=f"add{b}_{c}")
                for j in range(ndve):
                    nc.vector.tensor_scalar_add(
                        out=aslab[:, j, :], in0=WsT16_sb[:, c, :],
                        scalar1=UhT_sb[:, c, y0 + PY + j:y0 + PY + j + 1])
                nc.scalar.activation(out=tslab[:, PY:, :],
                                     in_=aslab[:], func=TANH)
            slabs.append(tslab)
        for j in range(GB):
            y = y0 + j
            for xc in range(XC):
                for c in range(EC):
                    nc.tensor.matmul(
                        out=eT_yh[y // P][:, xc, y % P:y % P + 1],
                        lhsT=slabs[c][:, j, xc * P:(xc + 1) * P],
                        rhs=V16_sb[:, c:c + 1],
                        start=False, stop=False,
                        skip_group_check=True)
        y0 += GB
        if y0 == P:
            _final_half(0)
    _final_half(1)

def _build():
    nc = bacc.Bacc("TRN2", target_bir_lowering=False, debug=False,
                   num_devices=NCORES)
    enc_d = nc.dram_tensor("enc", [Tx, E], F32, kind="ExternalInput").ap()
    dec_d = nc.dram_tensor("dec", [Ty, D], F32, kind="ExternalInput").ap()
    W_d = nc.dram_tensor("W", [E, E], F32, kind="ExternalInput").ap()
    U_d = nc.dram_tensor("U", [D, E], F32, kind="ExternalInput").ap()
    V_d = nc.dram_tensor("V", [E, 1], F32, kind="ExternalInput").ap()
    c_d = nc.dram_tensor("c_out", [Ty, E], F32, kind="ExternalOutput").ap()
    e_d = nc.dram_tensor("e_out", [Ty, Tx], F32, kind="ExternalOutput").ap()

    with tile.TileContext(nc) as tc:
        with ExitStack() as ctx:
            _build_body(tc, ctx, enc_d, dec_d, W_d, U_d, V_d, c_d, e_d)
    nc.compile()
    return nc


def _get_nc():
    global _NC
    if _NC is None:
        _NC = _build()
    return _NC


def kernel(encoder_out_seq, decoder_out_seq, W_a, U_a, V_a):
    enc = np.ascontiguousarray(np.asarray(encoder_out_seq, dtype=np.float32))
    dec = np.ascontiguousarray(np.asarray(decoder_out_seq, dtype=np.float32))
    W = np.ascontiguousarray(np.asarray(W_a, dtype=np.float32))
    U = np.ascontiguousarray(np.asarray(U_a, dtype=np.float32))
    V = np.ascontiguousarray(np.asarray(V_a, dtype=np.float32))

    nc = _get_nc()
    in_maps = [
        {"enc": enc[i], "dec": dec[i], "W": W, "U": U, "V": V}
        for i in range(NCORES)
    ]
    res = run_bass_kernel_spmd(nc, in_maps, list(range(NCORES)))
    global LAST_RESULTS
    LAST_RESULTS = res
    c = np.stack([res.results[i]["c_out"] for i in range(NCORES)])
    e = np.stack([res.results[i]["e_out"] for i in range(NCORES)])
    return c, e
